# revision 19
# baseline (speedup 1.0000x reference)
"""Multi-head self-attention with RoPE on 8 Trainium2 NeuronCores.

Full inputs in, full output out. Sharding: batch (2) x head-groups (4 heads
per core). Each core computes qkv projections for its heads, RoPE, full
softmax(QK^T)V, and a partial output projection; host sums the 4 partials
per batch element and adds b_out.

Problem shape: B=2, T=2048, D=1024, H=16, HD=64 (hardcoded).
"""

import numpy as np
from contextlib import ExitStack

import concourse.bass as bass
import concourse.mybir as mybir
import concourse.tile as tile
from concourse import bass_utils

B, T, D, H = 2, 2048, 1024, 16
HD = 64          # head dim
HL = 4           # heads per core
N_CORES = 8
ROPE_BASE = 10000.0

F32 = mybir.dt.float32
F32R = mybir.dt.float32r
BF16 = mybir.dt.bfloat16

Exp = mybir.ActivationFunctionType.Exp

# results of the last run (for test harness introspection)
LAST_RESULTS = None
TRACE = False


def _split_excess_waits(nc, cap=1):
    """walrus in this env rejects >1 sync-wait per instruction; split extras
    onto single-wait NoOps on the same engine queue."""
    n = 0
    for f in nc.m.functions:
        for bb in f.blocks:
            insts = bb.instructions
            if not any(
                i.sync_info is not None and len(i.sync_info.on_wait) > cap
                for i in insts
            ):
                continue
            out = []
            for inst in insts:
                si = inst.sync_info
                waits = list(si.on_wait) if si is not None else []
                if len(waits) > cap:
                    extra, keep = waits[:-cap], waits[-cap:]
                    for k, w in enumerate(extra):
                        nop = mybir.InstNoOp(
                            name=f"{inst.name}-ws{k}",
                            engine=inst.engine,
                            sync_info=mybir.SyncInfo(on_wait=[w], on_update=[]),
                            bass_nofuse=True,
                        )
                        nc.register_instruction(nop)
                        out.append(nop)
                        n += 1
                    inst.sync_info = mybir.SyncInfo(
                        on_wait=keep, on_update=list(si.on_update)
                    )
                out.append(inst)
            bb.instructions = out
    return n


def _build_bass(with_qkv_bias):
    nc = bass.Bass("TRN2", target_bir_lowering=False, debug=False, num_devices=1)

    # ---- DRAM I/O ----
    d_xT = nc.dram_tensor("xT", [D, T], F32R, kind="ExternalInput").ap()
    d_wqk = nc.dram_tensor("wqk", [D, 2 * HL * HD], F32R, kind="ExternalInput").ap()
    d_wv = nc.dram_tensor("wv", [D, HL * (HD + 1)], F32R, kind="ExternalInput").ap()
    d_bqk = nc.dram_tensor("bqk", [1, 2 * HL * HD], F32R, kind="ExternalInput").ap()
    d_bv = nc.dram_tensor("bv", [1, HL * (HD + 1)], F32R, kind="ExternalInput").ap()
    d_ones = nc.dram_tensor("ones", [1, T], F32R, kind="ExternalInput").ap()
    d_cos = nc.dram_tensor("cos2", [128, T], F32, kind="ExternalInput").ap()
    d_sin = nc.dram_tensor("sin2", [128, T], F32, kind="ExternalInput").ap()
    d_rT = nc.dram_tensor("rT", [128, 128], F32R, kind="ExternalInput").ap()
    d_ind = nc.dram_tensor("ind", [2, 128], F32R, kind="ExternalInput").ap()
    d_amask = nc.dram_tensor("amask", [128, T // 128], F32, kind="ExternalInput").ap()
    d_wo = nc.dram_tensor("wo", [HL * HD, D], F32R, kind="ExternalInput").ap()
    d_zeros = nc.dram_tensor("zeros", [HD, T], F32R, kind="ExternalInput").ap()
    d_out = nc.dram_tensor("out_part", [2, T, D], F32, kind="ExternalOutput").ap()

    NT = T // 128            # 16 token tiles
    NK = D // 128            # 8 contraction chunks
    SC = HD ** -0.5          # softmax scale

    with tile.TileContext(nc) as tc, ExitStack() as ctx:
        pool = lambda st, name, bufs: st.enter_context(tc.tile_pool(name=name, bufs=bufs))
        psum = lambda st, name, bufs: st.enter_context(
            tc.tile_pool(name=name, bufs=bufs, space="PSUM")
        )

        # lifetime-grouped pools: g_load dies after phase 2, g_att2/g_fin open late
        g_load = ctx.enter_context(ExitStack())
        psA = ctx.enter_context(ExitStack())

        p_const = pool(ctx, "const", 1)
        p_qkT = pool(ctx, "qkT", 2)
        p_kpad = pool(ctx, "kpad", 4)
        p_v = pool(ctx, "v", NT)
        p_xt = pool(g_load, "xt", NK)
        p_w = pool(g_load, "w", NK)
        p_wv = pool(g_load, "wv", NK)
        p_cs = pool(g_load, "cossin", 1)
        p_tmp = pool(g_load, "tmp", 2)

        ps_qk = psum(psA, "ps_qk", 2)
        ps_rot = psum(psA, "ps_rot", 2)
        ps_v = psum(psA, "ps_v", 2)

        # ---- weight / input loads ----
        xt = []
        wqk = []
        wv = []
        for k in range(NK):
            tk = p_xt.tile([128, T], F32R, tag="xt")
            for q4 in range(4):
                s4 = slice(q4 * 512, (q4 + 1) * 512)
                nc.sync.dma_start(tk[:, s4], d_xT[k * 128:(k + 1) * 128, s4])
            xt.append(tk)
            tw = p_w.tile([128, 2 * HL * HD], F32R, tag="wqk")
            nc.sync.dma_start(tw[:], d_wqk[k * 128:(k + 1) * 128, :])
            wqk.append(tw)
            tv = p_wv.tile([128, HL * (HD + 1)], F32R, tag="wv")
            nc.sync.dma_start(tv[:], d_wv[k * 128:(k + 1) * 128, :])
            wv.append(tv)

        # ---- constants / tables ----
        t_ones = p_const.tile([1, 512], F32R, tag="ones")
        nc.sync.dma_start(t_ones[:], d_ones[:, 0:512])
        t_bqk = p_const.tile([1, 2 * HL * HD], F32R, tag="bqk")
        nc.sync.dma_start(t_bqk[:], d_bqk[:])
        t_bv = p_const.tile([1, HL * (HD + 1)], F32R, tag="bv")
        nc.sync.dma_start(t_bv[:], d_bv[:])
        t_cos = p_cs.tile([128, T], F32, tag="cos")
        nc.sync.dma_start(t_cos[:], d_cos[:])
        t_sin = p_cs.tile([128, T], F32, tag="sin")
        nc.sync.dma_start(t_sin[:], d_sin[:])
        t_rT = p_const.tile([128, 128], F32R, tag="rT")
        nc.sync.dma_start(t_rT[:], d_rT[:])
        t_ind2 = p_const.tile([2, 128], F32R, tag="ind")
        nc.sync.dma_start(t_ind2[:], d_ind[:])
        t_amask = p_const.tile([128, T // 128], F32, tag="amask")
        nc.sync.dma_start(t_amask[:], d_amask[:])

        # ---- phase 1: q/k projections (feature-major) + RoPE ----
        # q chunks (c2=0,1) -> qkT[pair]; k chunks (c2=2,3) -> zero-padded
        # per-head tiles kpad[2*pair+{0,1}] so scores can run K=128.
        qkT = []
        kpad = []
        for pair in range(2):
            kA = p_kpad.tile([128, T], F32R, tag="kpad")
            kB = p_kpad.tile([128, T], F32R, tag="kpad")
            nc.sync.dma_start(kA[HD:128, :], d_zeros[:])
            nc.sync.dma_start(kB[0:HD, :], d_zeros[:])
            kpad.append((kA, kB))
        for c2 in range(4):
            is_k = c2 >= 2
            if not is_k:
                t_qk = p_qkT.tile([128, T], F32R, tag="qkT")
                qkT.append(t_qk)
            else:
                kA, kB = kpad[c2 - 2]
            for ih in range(4):  # quarters of the token axis
                TQ4 = T // 4
                sl = slice(ih * TQ4, (ih + 1) * TQ4)
                pqk = ps_qk.tile([128, TQ4], F32, tag="pqk")
                for k in range(NK):
                    nc.tensor.matmul(
                        pqk[:],
                        wqk[k][:, c2 * 128:(c2 + 1) * 128],
                        xt[k][:, sl],
                        start=(k == 0),
                        stop=(not with_qkv_bias and k == NK - 1),
                        skip_group_check=True,
                    )
                # bias (b_qkv slice) via K=1 matmul: adds bqk[f] to every token
                if with_qkv_bias:
                    nc.tensor.matmul(
                        pqk[:],
                        t_bqk[:, c2 * 128:(c2 + 1) * 128],
                        t_ones[:, 0:TQ4],
                        start=False,
                        stop=True,
                        skip_group_check=True,
                    )
                # RoPE: roped = raw*cos + R @ (raw*sin)   (sin is 32-symmetric)
                u_sb = p_tmp.tile([128, TQ4], F32R, tag="u")
                nc.vector.tensor_mul(u_sb[:], pqk[:], t_sin[:, sl])
                prot = ps_rot.tile([128, TQ4], F32, tag="prot")
                nc.tensor.matmul(
                    prot[:], t_rT[:], u_sb[:],
                    start=True, stop=True, skip_group_check=True,
                )
                c_sb = p_tmp.tile([128, TQ4], F32, tag="c")
                nc.vector.tensor_mul(c_sb[:], pqk[:], t_cos[:, sl])
                if not is_k:
                    nc.vector.tensor_add(t_qk[:, sl], c_sb[:], prot[:])
                else:
                    nc.vector.tensor_add(kA[0:HD, sl], c_sb[0:HD, :],
                                         prot[0:HD, :])
                    nc.vector.tensor_add(kB[HD:128, sl], c_sb[HD:128, :],
                                         prot[HD:128, :])

        # ---- phase 2: v projection (token-major, interleaved + ones col) ----
        VW = HL * (HD + 1)  # 260
        v_sb = []
        for t in range(NT):
            pv_ps = ps_v.tile([128, VW], F32, tag="pv_ps")
            for k in range(NK):
                nc.tensor.matmul(
                    pv_ps[:],
                    xt[k][:, t * 128:(t + 1) * 128],
                    wv[k][:],
                    start=(k == 0),
                    stop=False,
                    skip_group_check=True,
                )
            # bias + ones column (bv has 1.0 at the ones slots; always needed
            # for the softmax-sums ones column)
            nc.tensor.matmul(
                pv_ps[:], t_ones[:, 0:128], t_bv[:],
                start=False, stop=True, skip_group_check=True,
            )
            vt = p_v.tile([128, VW], BF16, tag="v")
            nc.vector.tensor_copy(vt[:], pv_ps[:])
            v_sb.append(vt)

        # ---- phase 3: attention, head pairs row-tiled on the PE array ----
        g_load.close()
        psA.close()
        psC = ctx.enter_context(ExitStack())
        ps_s = psum(psC, "ps_s", 2)
        ps_pv = psum(psC, "ps_pv", 1)
        ps_x = psum(psC, "ps_x", 1)
        p_e = pool(ctx, "eT", 4)
        p_a = pool(ctx, "aT", HL)
        p_fin = ctx.enter_context(ExitStack())
        p_anorm = pool(p_fin, "anorm", 2)
        p_wo = pool(p_fin, "wo", 2)
        p_osb = pool(p_fin, "osb", 2)
        p_small = pool(p_fin, "small", 1)
        wo_sb = []
        for c2 in range(2):
            wt = p_wo.tile([128, D], F32R, tag="wo")
            nc.sync.dma_start(wt[:], d_wo[c2 * 128:(c2 + 1) * 128, :])
            wo_sb.append(wt)
        a_sb = [None] * HL
        anorm = [None, None]
        TH2 = 1024
        for pair in range(2):
            hA, hB = 2 * pair, 2 * pair + 1
            qc = qkT[pair]
            atA = p_a.tile([HD + 1, T], F32, tag="aT")
            atB = p_a.tile([HD + 1, T], F32, tag="aT")
            a_sb[hA], a_sb[hB] = atA, atB
            NTT = T // 128
            sums128 = p_small.tile([128, 2 * NTT], F32, tag=f"sums{pair}")
            for hh in range(2):
                h = 2 * pair + hh
                at = (atA, atB)[hh]
                kp = kpad[pair][hh]
                for ih in range(2):
                    qsl = slice(ih * TH2, (ih + 1) * TH2)
                    pv = ps_pv.tile([HD + 1, TH2], F32, tag="pv")
                    for jb in range(NT):
                        s_ps = ps_s.tile([128, TH2], F32, tag="sT")
                        jsl = slice(jb * 128, (jb + 1) * 128)
                        for n5 in range(2):
                            s5 = slice(n5 * 512, (n5 + 1) * 512)
                            g5 = slice(ih * TH2 + n5 * 512,
                                       ih * TH2 + (n5 + 1) * 512)
                            nc.tensor.matmul(
                                s_ps[:, s5], kp[:, jsl], qc[:, g5],
                                start=True, stop=True, skip_group_check=True,
                            )
                        e_sb = p_e.tile([128, TH2], BF16, tag="eT")
                        nc.scalar.activation(e_sb[:], s_ps[:], Exp,
                                             bias=t_amask[:, jb:jb + 1],
                                             scale=SC)
                        for n5 in range(2):
                            s5 = slice(n5 * 512, (n5 + 1) * 512)
                            nc.tensor.matmul(
                                pv[:, s5],
                                v_sb[jb][:, h * (HD + 1):(h + 1) * (HD + 1)],
                                e_sb[:, s5],
                                start=(jb == 0), stop=(jb == NT - 1),
                                skip_group_check=True,
                            )
                    nc.vector.tensor_copy(at[:, qsl], pv[:])
            for i, at in enumerate((atA, atB)):
                nc.sync.dma_start(
                    sums128[:, i * NTT:(i + 1) * NTT],
                    at[HD:HD + 1, :].rearrange("o (p c) -> o p c", p=128),
                )
            # normalization for this pair (overlaps the next pair's attention):
            # gather the two sums rows into [128, 32] (p-major), reciprocal,
            # scatter back to [2, T], broadcast via K=2 indicator matmul,
            # multiply the pair's aT rows into the stacked+normalized chunk.
            for i, at in enumerate((atA, atB)):
                nc.sync.dma_start(
                    sums128[:, i * NTT:(i + 1) * NTT],
                    at[HD:HD + 1, :].rearrange("o (p c) -> o p c", p=128),
                )
            recip128 = p_small.tile([128, 2 * NTT], F32, tag=f"recip{pair}")
            nc.vector.reciprocal(recip128[:], sums128[:])
            recip2 = p_small.tile([2, T], F32R, tag=f"recip2_{pair}")
            for i in range(2):
                nc.sync.dma_start(
                    recip2[i:i + 1, :].rearrange("o (p c) -> o p c", p=128),
                    recip128[:, i * NTT:(i + 1) * NTT].bitcast(F32R),
                )
            ar = p_anorm.tile([128, T], F32, tag="anorm_raw")
            nc.sync.dma_start(ar[0:HD, :], atA[0:HD, :])
            nc.sync.dma_start(ar[HD:2 * HD, :], atB[0:HD, :])
            an = p_anorm.tile([128, T], F32R, tag="anorm")
            for ibh in range(2):
                hsl = slice(ibh * (T // 2), (ibh + 1) * (T // 2))
                pb = ps_x.tile([128, T // 2], F32, tag="px")
                for n5 in range(2):
                    s5 = slice(n5 * 512, (n5 + 1) * 512)
                    g5 = slice(ibh * (T // 2) + n5 * 512,
                               ibh * (T // 2) + (n5 + 1) * 512)
                    nc.tensor.matmul(
                        pb[:, s5], t_ind2[:], recip2[:, g5],
                        start=True, stop=True, skip_group_check=True,
                    )
                nc.vector.tensor_mul(an[:, hsl], pb[:], ar[:, hsl])
            anorm[pair] = an

            # partial output projection for this pair's heads. Pair 0 runs
            # under pair 1's attention (single psum slot, copies on DVE);
            # pair 1 (epilog) alternates two slots with copies on ACT.
            for t in range(NT):
                if pair == 0:
                    pp = ps_x.tile([128, D], F32, tag="px")
                else:
                    pp = (ps_x if t % 2 == 0 else ps_pv).tile(
                        [128, D], F32, tag=("px" if t % 2 == 0 else "pv"))
                for n5 in range(2):
                    s5 = slice(n5 * 512, (n5 + 1) * 512)
                    nc.tensor.matmul(
                        pp[:, s5],
                        an[:, t * 128:(t + 1) * 128],
                        wo_sb[pair][:, s5],
                        start=True, stop=True, skip_group_check=True,
                    )
                osb = p_osb.tile([128, D], F32, tag="osb")
                if pair == 0:
                    nc.vector.tensor_copy(osb[:], pp[:])
                else:
                    nc.scalar.copy(osb[:], pp[:])
                nc.sync.dma_start(d_out[pair, t * 128:(t + 1) * 128, :], osb[:])

        # ---- phase 4 (tail): per-pair normalization started inside phase 3 ----
    _split_excess_waits(nc)
    return nc


_NC_CACHE = {}


def _rope_tables():
    inv_freq = (1.0 / (ROPE_BASE ** (np.arange(0, HD, 2, dtype=np.float32) / HD))
                ).astype(np.float32)
    t = np.arange(T, dtype=np.float32)
    freqs = np.einsum("t,f->tf", t, inv_freq).astype(np.float32)  # (T, HD/2)
    emb = np.concatenate([freqs, freqs], axis=-1)                  # (T, HD)
    cosT = np.cos(emb).astype(np.float32).T                        # (HD, T)
    sinT = np.sin(emb).astype(np.float32).T
    cos2 = np.ascontiguousarray(np.tile(cosT, (2, 1)))             # (128, T)
    sin2 = np.ascontiguousarray(np.tile(sinT, (2, 1)))
    return cos2, sin2


def _rot_matrix():
    r = np.zeros((128, 128), dtype=np.float32)
    for p0 in (0, 64):
        for d in range(32):
            r[p0 + d, p0 + 32 + d] = -1.0
            r[p0 + 32 + d, p0 + d] = 1.0
    return np.ascontiguousarray(r.T)


def kernel(x, W_qkv, b_qkv, W_out, b_out, padding_mask):
    global _NC_CACHE, LAST_RESULTS
    x = np.asarray(x, dtype=np.float32)
    W_qkv = np.asarray(W_qkv, dtype=np.float32)
    b_qkv = np.asarray(b_qkv, dtype=np.float32)
    W_out = np.asarray(W_out, dtype=np.float32)
    b_out = np.asarray(b_out, dtype=np.float32)
    padding_mask = np.asarray(padding_mask)

    with_qkv_bias = bool(np.any(b_qkv[:2 * D]))
    if with_qkv_bias not in _NC_CACHE:
        _NC_CACHE[with_qkv_bias] = _build_bass(with_qkv_bias)
    nc = _NC_CACHE[with_qkv_bias]

    cos2, sin2 = _rope_tables()
    rT = _rot_matrix()

    ind = np.zeros((2, 128), dtype=np.float32)
    for f in range(128):
        ind[f // 64, f] = 1.0

    ones = np.ones((1, T), dtype=np.float32)

    in_maps = []
    for c in range(N_CORES):
        b = c // 4
        g = c % 4
        q0 = g * HL * HD
        wq = W_qkv[:, q0:q0 + HL * HD]
        wk = W_qkv[:, D + q0:D + q0 + HL * HD]
        wv_flat = W_qkv[:, 2 * D + q0:2 * D + q0 + HL * HD]
        # interleave v columns with a zero (ones-slot) column per head
        wv_aug = np.zeros((D, HL * (HD + 1)), dtype=np.float32)
        bv_aug = np.zeros((1, HL * (HD + 1)), dtype=np.float32)
        for h in range(HL):
            wv_aug[:, h * (HD + 1):h * (HD + 1) + HD] = wv_flat[:, h * HD:(h + 1) * HD]
            bv_aug[0, h * (HD + 1):h * (HD + 1) + HD] = \
                b_qkv[2 * D + q0 + h * HD:2 * D + q0 + (h + 1) * HD]
            bv_aug[0, h * (HD + 1) + HD] = 1.0
        bqk = np.concatenate(
            [b_qkv[q0:q0 + HL * HD], b_qkv[D + q0:D + q0 + HL * HD]]
        ).reshape(1, -1).astype(np.float32)
        amask = np.where(padding_mask[b], np.float32(-1e30), np.float32(0.0))
        amask = np.ascontiguousarray(amask.reshape(T // 128, 128).T.astype(np.float32))
        in_maps.append({
            "xT": np.ascontiguousarray(x[b].T),
            "wqk": np.ascontiguousarray(np.concatenate([wq, wk], axis=1)),
            "wv": wv_aug,
            "bqk": bqk,
            "bv": bv_aug,
            "ones": ones,
            "cos2": cos2,
            "sin2": sin2,
            "rT": rT,
            "ind": ind,
            "amask": amask,
            "wo": np.ascontiguousarray(W_out[q0:q0 + HL * HD, :]),
            "zeros": np.zeros((HD, T), dtype=np.float32),
        })

    res = bass_utils.run_bass_kernel_spmd(
        nc, in_maps, core_ids=list(range(N_CORES)), trace=TRACE,
    )
    LAST_RESULTS = res

    out = np.zeros((B, T, D), dtype=np.float64)
    for c in range(N_CORES):
        p = res.results[c]["out_part"].astype(np.float64)
        out[c // 4] += p[0] + p[1]
    out += b_out.astype(np.float64)
    return out.astype(np.float32)
